# revision 13
# baseline (speedup 1.0000x reference)
"""Trainium2 Bass kernel for nn_AdaptiveSubgraphLayer (GNN message passing).

Strategy (8 NeuronCores, SPMD, no collectives needed):
  - Nodes are partitioned across cores by batch (B=32 -> 4 batches/core); each
    edge follows its destination node. Segment statistics, the per-batch center
    user lookup, scoring and top-K pruning are then all core-local.
  - Within a core, nodes are ordered by (degree, local batch); each node's
    edges occupy contiguous "slots" in that order. All index manipulation is
    done on the host at build time and baked into the program; all float math
    runs on device. The host equalizes per-(degree,batch) cell sizes across
    cores (dummy nodes/slots) so one SPMD program serves all 8 cores.
  - GRU messages: hs rows fetched by a transposed dma_gather (bf16) giving
    [D, slots]; gh = hs @ W_h on TensorE (the reference's per-edge [E,3D]
    matmul only has 20000 unique rows); gi = rel_table@W_i+b via a 27-row
    one-hot matmul. Gates on ScalarE, elementwise on VectorE.
  - PNA stats: mean/var via bn_stats on per-node slot groups; min/max via
    strided halving trees (degree-homogeneous cells -> uniform strides);
    deg-derived scalers (amp/att) host-precomputed, applied post-matmul.
  - h_tildeT accumulated in PSUM (13 K=128 matmuls / 512-node window).
  - Scoring: logit rows via M=1 matmuls; the center-user term enters as a
    per-partition ACT bias in batch-row layout; top-K=50 via max8+match_replace.
"""

import numpy as np
import ml_dtypes

import concourse.bass as bass
import concourse.bacc as bacc
import concourse.tile as tile
import concourse.mybir as mybir
from concourse import tile_utils
from concourse.bass_utils import run_bass_kernel_spmd

bf16 = ml_dtypes.bfloat16

P = 128
D = 128
B = 32
N_CORES = 8
BPC = B // N_CORES
N_USER, N_ITEM = 50_000, 30_000
K_TOP = 50
ITEM_BONUS = 0.05
TAU = 1.0
DELTA = 1.0
SLAB = 2048          # slots per dma_gather / m-ring tile
GTILE = 512          # slots per GRU psum tile
WIN = 512            # nodes per PNA window
STD_EPS_DEFAULT = float(np.sqrt(np.float32(1e-5)))
NEG = -10.0

tile_utils.max_sbuf_usage = 208 * 1024


# ================================================================ host layout


def _plan(inputs):
    q_sub = np.asarray(inputs["q_sub"]).astype(np.int64)
    hidden = np.asarray(inputs["hidden"], dtype=np.float32)
    edges = np.asarray(inputs["edges"]).astype(np.int64)
    nodes = np.asarray(inputs["nodes"]).astype(np.int64)
    old_idx = np.asarray(inputs["old_nodes_new_idx"]).astype(np.int64)
    rel_table = np.asarray(inputs["rel_table"], dtype=np.float32)
    W_i = np.asarray(inputs["W_i"], dtype=np.float32)
    W_h = np.asarray(inputs["W_h"], dtype=np.float32)
    b_i = np.asarray(inputs["b_i"], dtype=np.float32)
    b_h = np.asarray(inputs["b_h"], dtype=np.float32)
    W_pna = np.asarray(inputs["W_pna"], dtype=np.float32)
    b_pna = np.asarray(inputs["b_pna"], dtype=np.float32)
    W_score = np.asarray(inputs["W_score"], dtype=np.float32)
    b_score = np.asarray(inputs["b_score"], dtype=np.float32)
    id_layer = int(inputs["id_layer"])
    n_layer = int(inputs["n_layer"])

    N = nodes.shape[0]
    NP_ = hidden.shape[0]
    NR = rel_table.shape[0]

    node_batch = nodes[:, 0]
    node_ent = nodes[:, 1]
    sub, rel, obj = edges[:, 4], edges[:, 2], edges[:, 5]
    deg = np.bincount(obj, minlength=N).astype(np.int64)
    owner = node_batch // BPC
    lbat = node_batch % BPC

    classes = [int(d) for d in np.unique(deg)]
    counts = np.zeros((N_CORES, len(classes), BPC), dtype=np.int64)
    node_ids = [[[None] * BPC for _ in classes] for _ in range(N_CORES)]
    for c in range(N_CORES):
        sel = np.nonzero(owner == c)[0]
        dsel, lsel = deg[sel], lbat[sel]
        for di, d in enumerate(classes):
            sd = sel[dsel == d]
            ls = lsel[dsel == d]
            for b in range(BPC):
                ids = np.sort(sd[ls == b])
                node_ids[c][di][b] = ids
                counts[c, di, b] = len(ids)
    cap = counts.max(axis=0)

    # ---- cells (uniform layout), node columns
    cells = []
    col = 0
    for di, d in enumerate(classes):
        for b in range(BPC):
            n = int(cap[di, b])
            if n == 0:
                continue
            cells.append(dict(d=d, lb=b, col=col, n=n))
            col += n
    n_cols = col
    NODE_CAP = -(-n_cols // WIN) * WIN
    NW = NODE_CAP // WIN

    # ---- slot layout with slab/window-aligned pieces
    # piece: (d, col, slot, n) — n nodes of degree d, node cols [col, col+n),
    # slots [slot, slot+n*d); never crosses a slab or window boundary.
    pieces = []
    slot = 0
    for cell in cells:
        d, n_left, ccol = cell["d"], cell["n"], cell["col"]
        if d == 0:
            continue
        cell["pieces"] = []
        while n_left > 0:
            nb = (slot // SLAB + 1) * SLAB  # next slab boundary
            fit_slab = (nb - slot) // d
            if fit_slab == 0:
                slot = nb
                continue
            fit_win = (ccol // WIN + 1) * WIN - ccol
            take = min(n_left, fit_slab, fit_win)
            pc = dict(d=d, col=ccol, slot=slot, n=take)
            pieces.append(pc)
            cell["pieces"].append(pc)
            ccol += take
            slot += take * d
            n_left -= take
    n_slots = slot
    SLOT_CAP = -(-slot // SLAB) * SLAB
    n_slabs = SLOT_CAP // SLAB

    # ---- topk segments: per lb, (node col, n, row offset)
    segs = [[] for _ in range(BPC)]
    rowoff = [0] * BPC
    for cell in cells:
        b = cell["lb"]
        segs[b].append(dict(col=cell["col"], n=cell["n"], off=rowoff[b]))
        rowoff[b] += cell["n"]
    TCAP = max(16, -(-max(rowoff) // 8) * 8)

    # topk assembly pieces: (window, lb, col, off, n) never crossing windows
    tkpieces = []
    for bl in range(BPC):
        for sg in segs[bl]:
            c0, o0, left = sg["col"], sg["off"], sg["n"]
            while left > 0:
                fit = (c0 // WIN + 1) * WIN - c0
                take = min(left, fit)
                tkpieces.append(dict(w=c0 // WIN, lb=bl, col=c0, off=o0, n=take))
                c0 += take
                o0 += take
                left -= take

    # ---- per-core fills
    node_of_col = -np.ones((N_CORES, NODE_CAP), dtype=np.int64)
    slot_sub = np.zeros((N_CORES, SLOT_CAP), dtype=np.int64)
    slot_rel = -np.ones((N_CORES, SLOT_CAP), dtype=np.int64)

    eorder = np.argsort(obj, kind="stable")
    estart = np.zeros(N + 1, dtype=np.int64)
    np.cumsum(np.bincount(obj, minlength=N), out=estart[1:])
    dmap = {d: i for i, d in enumerate(classes)}

    for c in range(N_CORES):
        for cell in cells:
            di = dmap[cell["d"]]
            ids = node_ids[c][di][cell["lb"]]
            node_of_col[c, cell["col"]:cell["col"] + len(ids)] = ids
            d = cell["d"]
            if d == 0 or len(ids) == 0:
                continue
            ee = np.concatenate([eorder[estart[g]:estart[g + 1]] for g in ids])
            fsub, frel = sub[ee], rel[ee]
            done = 0  # nodes of this cell already placed
            for pc in cell["pieces"]:
                k = min(max(len(ids) - done, 0), pc["n"])
                if k > 0:
                    a = done * d
                    slot_sub[c, pc["slot"]:pc["slot"] + k * d] = fsub[a:a + k * d]
                    slot_rel[c, pc["slot"]:pc["slot"] + k * d] = frel[a:a + k * d]
                done += pc["n"]

    # ---- host-side derived data
    prev_of_node = -np.ones(N, dtype=np.int64)
    prev_of_node[old_idx] = np.arange(NP_)

    center = np.zeros(B, dtype=np.int64)
    for b in range(B):
        m = (node_batch == b) & (node_ent == q_sub[b])
        center[b] = int(np.argmax(m))

    def core_inputs(c):
        nid = node_of_col[c]
        ok = nid >= 0
        nclip = np.clip(nid, 0, None)
        dg = np.where(ok, deg[nclip], 0).astype(np.float32)
        logd = np.log1p(dg)
        amp = (logd / DELTA).astype(np.float32).reshape(1, NODE_CAP)
        att = (DELTA / (logd + 1.0)).astype(np.float32).reshape(1, NODE_CAP)

        hprevT = np.zeros((P, NODE_CAP), dtype=np.float32)
        pn = np.where(ok, prev_of_node[nclip], -1)
        sel = pn >= 0
        hprevT[:, sel] = hidden[pn[sel]].T

        ohb = np.zeros((BPC, NODE_CAP), dtype=np.float32)
        lbc = np.where(ok, lbat[nclip], 0)
        for bl in range(BPC):
            ohb[bl, ok & (lbc == bl)] = 1.0

        ctr = np.zeros((BPC, NODE_CAP), dtype=np.float32)
        colmap = -np.ones(N, dtype=np.int64)
        colmap[nid[ok]] = np.nonzero(ok)[0]
        for bl in range(BPC):
            cc = colmap[center[c * BPC + bl]]
            assert cc >= 0, "center node must be on its batch's core"
            ctr[bl, cc] = 1.0

        idx32 = np.ascontiguousarray(
            slot_sub[c].reshape(SLOT_CAP // P, P).T.astype(np.int32)
        )

        ohrel = np.zeros((NR, SLOT_CAP), dtype=bf16)
        sr = slot_rel[c]
        v = sr >= 0
        ohrel[sr[v], np.nonzero(v)[0]] = 1.0

        prep = np.full((BPC, TCAP), NEG, dtype=np.float32)
        is_item = ok & (node_ent[nclip] >= N_USER) & (node_ent[nclip] < N_USER + N_ITEM)
        cand = ok & ~(pn >= 0)
        candval = np.where(is_item, 1.0 + ITEM_BONUS, 1.0).astype(np.float32)
        for bl in range(BPC):
            for sg in segs[bl]:
                cs = slice(sg["col"], sg["col"] + sg["n"])
                prep[bl, sg["off"]:sg["off"] + sg["n"]] = np.where(
                    cand[cs], candval[cs], NEG
                )

        # cols 0-3: Ws1 x4 (v-rows), col 4: Ws2, cols 5-8: Ws2 x4 (topk rows)
        wsc5 = np.zeros((P, 9), dtype=bf16)
        for k in range(4):
            wsc5[:, k] = W_score[:128, 0].astype(bf16)
            wsc5[:, 5 + k] = W_score[128:, 0].astype(bf16)
        wsc5[:, 4] = W_score[128:, 0].astype(bf16)

        return dict(
            hsb=np.ascontiguousarray(hidden),
            idx32=idx32,
            ohrel=np.ascontiguousarray(ohrel),
            whf=np.ascontiguousarray(W_h),
            gib=np.ascontiguousarray((rel_table @ W_i + b_i + b_h).astype(bf16)),
            wpna=np.ascontiguousarray(
                W_pna.reshape(13, 128, 128).transpose(1, 0, 2).reshape(128, 13 * 128)
            ),
            bpna=np.ascontiguousarray(b_pna.reshape(128, 1)),
            wsc5=wsc5,
            identf=np.eye(128, dtype=np.float32),
            identb=np.eye(128, dtype=bf16),
            ones128=np.ones((1, 128), dtype=np.float32),
            ampr=amp, attr=att, hprevT=hprevT, ohb=ohb, ctr=ctr, prep=prep,
        )

    meta = dict(
        N=N, NP=NP_, NR=NR,
        NODE_CAP=NODE_CAP, SLOT_CAP=SLOT_CAP, TCAP=TCAP, NW=NW,
        n_slabs=n_slabs, n_slots=n_slots, n_cols=n_cols,
        pieces=pieces, segs=segs, cells=cells, tkpieces=tkpieces,
        b_score=float(b_score[0]),
        do_topk=(id_layer < n_layer - 1),
    )
    host = dict(
        node_of_col=node_of_col, prev_of_node=prev_of_node,
        node_ent=node_ent, old_idx=old_idx,
        core_inputs=core_inputs,
        id_layer=id_layer, n_layer=n_layer,
    )
    return meta, host


# ================================================================ bass build


def _build(meta):
    dt = mybir.dt
    Alu = mybir.AluOpType
    Act = mybir.ActivationFunctionType
    NODE_CAP = meta["NODE_CAP"]
    SLOT_CAP = meta["SLOT_CAP"]
    TCAP = meta["TCAP"]
    NP_ = meta["NP"]
    NR = meta["NR"]
    NW = meta["NW"]
    n_slabs = meta["n_slabs"]
    b_score = meta["b_score"]

    nc = bacc.Bacc("TRN2", target_bir_lowering=False, debug=False)

    def din(name, shape, dty):
        return nc.declare_dram_parameter(name, list(shape), dty, isOutput=False)

    hsb = din("hsb", [NP_, D], dt.float32)
    idx32 = din("idx32", [P, SLOT_CAP // P], dt.int32)
    ohrel = din("ohrel", [NR, SLOT_CAP], dt.bfloat16)
    whf = din("whf", [P, 3 * D], dt.float32)
    gib = din("gib", [NR, 3 * D], dt.bfloat16)
    wpna = din("wpna", [P, 13 * P], dt.float32)
    bpna = din("bpna", [P, 1], dt.float32)
    wsc5 = din("wsc5", [P, 9], dt.bfloat16)
    identf = din("identf", [P, P], dt.float32)
    identb = din("identb", [P, P], dt.bfloat16)
    ones128 = din("ones128", [1, P], dt.float32)
    ampr = din("ampr", [1, NODE_CAP], dt.float32)
    attr = din("attr", [1, NODE_CAP], dt.float32)
    hprevT = din("hprevT", [P, NODE_CAP], dt.float32)
    ohb = din("ohb", [BPC, NODE_CAP], dt.float32)
    ctr = din("ctr", [BPC, NODE_CAP], dt.float32)
    prep = din("prep", [BPC, TCAP], dt.float32)

    out_hall = nc.declare_dram_parameter("out_hall", [P, NODE_CAP], dt.float32, isOutput=True)
    out_alpha = nc.declare_dram_parameter("out_alpha", [1, NODE_CAP], dt.float32, isOutput=True)
    out_keep = nc.declare_dram_parameter("out_keep", [BPC, TCAP], dt.float32, isOutput=True)
    lg1_dram = nc.dram_tensor("lg1_scratch", [1, NODE_CAP], dt.float32)

    # group pieces by slab and precompute window transitions
    slab_pieces = [[] for _ in range(n_slabs)]
    for pc in meta["pieces"]:
        slab_pieces[pc["slot"] // SLAB].append(pc)

    with tile.TileContext(nc) as tc:
        with (
            tc.tile_pool(name="persist", bufs=1) as pp,
            tc.tile_pool(name="slabio", bufs=2) as sp,
            tc.tile_pool(name="gru", bufs=2) as gp,
            tc.tile_pool(name="stats", bufs=2) as stp,
            tc.tile_pool(name="bnp", bufs=1) as bnp,
            tc.tile_pool(name="rows", bufs=2) as rp,
            tc.tile_pool(name="rows1", bufs=1) as r1,
            tc.tile_pool(name="misc1", bufs=1) as m1,
            tc.tile_pool(name="gpsum", bufs=1, space="PSUM") as gps,
            tc.tile_pool(name="ppsum", bufs=1, space="PSUM") as pps,
            tc.tile_pool(name="mpsum", bufs=1, space="PSUM") as mps,
        ):
            # ---- persistent
            htT = pp.tile([P, NODE_CAP], dt.bfloat16, tag="htT")
            cu_acc = pp.tile([BPC, 1], dt.float32, tag="cu_acc")
            nc.vector.memset(cu_acc[:], 0.0)
            eps_c = pp.tile([P, 1], dt.float32, tag="eps_c")
            nc.vector.memset(eps_c[:], 1e-5)
            bsc_c = pp.tile([P, 1], dt.float32, tag="bsc_c")
            nc.vector.memset(bsc_c[:], b_score / TAU)

            whf_s = pp.tile([P, 3 * D], dt.float32, tag="whf")
            gib_s = pp.tile([NR, 3 * D], dt.bfloat16, tag="gib")
            wpna_s = pp.tile([P, 13 * P], dt.float32, tag="wpna")
            bpna_s = pp.tile([P, 1], dt.float32, tag="bpna")
            wsc5_s = pp.tile([P, 9], dt.bfloat16, tag="wsc5")
            identf_s = pp.tile([P, P], dt.float32, tag="identf")
            identb_s = pp.tile([P, P], dt.bfloat16, tag="identb")
            ones_s = pp.tile([1, P], dt.float32, tag="ones128")
            prep_s = pp.tile([BPC, TCAP], dt.float32, tag="prep")
            for t, src in [
                (whf_s, whf), (gib_s, gib), (wpna_s, wpna), (bpna_s, bpna),
                (wsc5_s, wsc5), (identf_s, identf), (identb_s, identb),
                (ones_s, ones128), (prep_s, prep),
            ]:
                nc.sync.dma_start(out=t[:], in_=src[:])

            # window state
            cur_win = -1
            win_tiles = None

            def wopen(w):
                nonlocal win_tiles
                mean_t = stp.tile([P, WIN], dt.float32, tag="mean")
                std_t = stp.tile([P, WIN], dt.float32, tag="std")
                mn_t = stp.tile([P, WIN], dt.float32, tag="mn")
                mx_t = stp.tile([P, WIN], dt.float32, tag="mx")
                nc.gpsimd.memset(mean_t[:], 0.0)
                nc.gpsimd.memset(std_t[:], STD_EPS_DEFAULT)
                nc.gpsimd.memset(mn_t[:], 0.0)
                nc.gpsimd.memset(mx_t[:], 0.0)
                win_tiles = (mean_t, std_t, mn_t, mx_t)

            def wclose(w):
                mean_t, std_t, mn_t, mx_t = win_tiles
                w0 = w * WIN
                y1 = pps.tile([P, WIN], dt.float32, tag="y1")
                y2 = pps.tile([P, WIN], dt.float32, tag="y23")
                hpv = stp.tile([P, WIN], dt.float32, tag="hpv")
                nc.sync.dma_start(out=hpv[:], in_=hprevT[:, w0:w0 + WIN])
                stats = (mean_t, std_t, mn_t, mx_t)
                for k in range(4):
                    nc.tensor.matmul(
                        y1[:], wpna_s[:, k * P:(k + 1) * P], stats[k][:],
                        start=(k == 0), stop=False,
                    )
                nc.tensor.matmul(
                    y1[:], wpna_s[:, 12 * P:13 * P], hpv[:], start=False, stop=False
                )
                ampv = rp.tile([1, WIN], dt.float32, tag="ampv")
                attv = rp.tile([1, WIN], dt.float32, tag="attv")
                nc.sync.dma_start(out=ampv[:], in_=ampr[:, w0:w0 + WIN])
                nc.sync.dma_start(out=attv[:], in_=attr[:, w0:w0 + WIN])
                for k in range(4):
                    nc.tensor.matmul(
                        y2[:], wpna_s[:, (4 + k) * P:(5 + k) * P], stats[k][:],
                        start=(k == 0), stop=(k == 3),
                    )
                y2c = m1.tile([P, WIN], dt.float32, tag="y2c")
                nc.scalar.activation(y2c[:], y2[:], Act.Copy)
                y3 = pps.tile([P, WIN], dt.float32, tag="y23")
                for k in range(4):
                    nc.tensor.matmul(
                        y3[:], wpna_s[:, (8 + k) * P:(9 + k) * P], stats[k][:],
                        start=(k == 0), stop=(k == 3),
                    )
                y3c = m1.tile([P, WIN], dt.float32, tag="y3c")
                nc.scalar.activation(y3c[:], y3[:], Act.Copy)
                y2s = m1.tile([P, WIN], dt.float32, tag="y2s")
                y3s = m1.tile([P, WIN], dt.float32, tag="y3s")
                mp = mps.tile([P, WIN], dt.float32, tag="mp")
                nc.tensor.matmul(mp[:], ones_s[:], ampv[:], start=True, stop=True)
                nc.vector.tensor_tensor(y2s[:], mp[:], y2c[:], op=Alu.mult)
                mp2 = mps.tile([P, WIN], dt.float32, tag="mp")
                nc.tensor.matmul(mp2[:], ones_s[:], attv[:], start=True, stop=True)
                nc.vector.tensor_tensor(y3s[:], mp2[:], y3c[:], op=Alu.mult)
                nc.tensor.matmul(y1[:], identf_s[:], y2s[:], start=False, stop=False)
                nc.tensor.matmul(y1[:], identf_s[:], y3s[:], start=False, stop=True)
                nc.scalar.activation(
                    htT[:, w0:w0 + WIN], y1[:], Act.Identity, bias=bpna_s[:, 0:1]
                )
                # v rows (Ws1 x4) + cu accumulation against center one-hot
                mpv = mps.tile([P, WIN], dt.float32, tag="mp")
                nc.tensor.matmul(
                    mpv[0:BPC, :], wsc5_s[:, 0:BPC], htT[:, w0:w0 + WIN],
                    start=True, stop=True,
                )
                ctrv = r1.tile([BPC, WIN], dt.float32, tag="ctrv")
                nc.sync.dma_start(out=ctrv[:], in_=ctr[:, w0:w0 + WIN])
                prodw = r1.tile([BPC, WIN], dt.float32, tag="prodw")
                nc.vector.tensor_tensor(prodw[:], mpv[0:BPC, :], ctrv[:], op=Alu.mult)
                cuw = r1.tile([BPC, 1], dt.float32, tag="cuw")
                nc.vector.tensor_reduce(cuw[:], prodw[:], mybir.AxisListType.X, Alu.add)
                nc.vector.tensor_tensor(cu_acc[:], cu_acc[:], cuw[:], op=Alu.add)
                # logit part1 = Ws2 . h_t  -> DRAM scratch row
                mpl = mps.tile([P, WIN], dt.float32, tag="mp")
                nc.tensor.matmul(
                    mpl[0:1, :], wsc5_s[:, 4:5], htT[:, w0:w0 + WIN],
                    start=True, stop=True,
                )
                lgv = r1.tile([1, WIN], dt.float32, tag="lgv")
                nc.vector.tensor_copy(lgv[:], mpl[0:1, :])
                nc.sync.dma_start(out=lg1_dram[:, w0:w0 + WIN], in_=lgv[:])

            # ---------------- main slab loop with interleaved stats windows
            CPS = SLAB // P  # gather chunks per slab
            for sl in range(n_slabs):
                s0 = sl * SLAB
                idx_t = sp.tile([P, CPS], dt.int32, tag="idx")
                nc.sync.dma_start(
                    out=idx_t[:], in_=idx32[:, sl * CPS:(sl + 1) * CPS]
                )
                hsT = sp.tile([P, SLAB], dt.float32, tag="hsT")
                for t in range(SLAB // GTILE):
                    tp = mps.tile([P, GTILE], dt.float32, tag="tps")
                    for q in range(GTILE // P):
                        j = t * (GTILE // P) + q
                        g = sp.tile([P, P], dt.float32, tag="gat")
                        nc.gpsimd.indirect_dma_start(
                            out=g[:], out_offset=None, in_=hsb[:],
                            in_offset=bass.IndirectOffsetOnAxis(
                                ap=idx_t[:, j:j + 1], axis=0
                            ),
                        )
                        nc.tensor.transpose(
                            tp[:, q * P:(q + 1) * P], g[:], identf_s[:]
                        )
                    nc.scalar.activation(
                        hsT[:, t * GTILE:(t + 1) * GTILE], tp[:], Act.Copy
                    )
                m_sl = sp.tile([P, SLAB], dt.bfloat16, tag="m_sl")

                for t in range(SLAB // GTILE):
                    a = t * GTILE
                    hsv = hsT[:, a:a + GTILE]
                    oh_t = sp.tile([NR, GTILE], dt.bfloat16, tag="ohrel")
                    nc.sync.dma_start(out=oh_t[:], in_=ohrel[:, s0 + a:s0 + a + GTILE])
                    ohv = oh_t[:]
                    ppr = gps.tile([P, GTILE], dt.float32, tag="ppr")
                    ppz = gps.tile([P, GTILE], dt.float32, tag="ppz")
                    phn = gps.tile([P, GTILE], dt.float32, tag="phn")
                    pin = gps.tile([P, GTILE], dt.float32, tag="pin")
                    nc.tensor.matmul(ppr[:], whf_s[:, 0:D], hsv, start=True, stop=False)
                    nc.tensor.matmul(ppr[:], gib_s[:, 0:D], ohv, start=False, stop=True)
                    nc.tensor.matmul(ppz[:], whf_s[:, D:2 * D], hsv, start=True, stop=False)
                    nc.tensor.matmul(ppz[:], gib_s[:, D:2 * D], ohv, start=False, stop=True)
                    nc.tensor.matmul(phn[:], whf_s[:, 2 * D:3 * D], hsv, start=True, stop=True)

                    r_t = gp.tile([P, GTILE], dt.bfloat16, tag="r")
                    z_t = gp.tile([P, GTILE], dt.bfloat16, tag="z")
                    nc.scalar.activation(r_t[:], ppr[:], Act.Sigmoid)
                    nc.scalar.activation(z_t[:], ppz[:], Act.Sigmoid)

                    rh_t = gp.tile([P, GTILE], dt.bfloat16, tag="rh")
                    nc.vector.tensor_tensor(rh_t[:], r_t[:], phn[:], op=Alu.mult)

                    nc.tensor.matmul(pin[:], gib_s[:, 2 * D:3 * D], ohv, start=True, stop=False)
                    nc.tensor.matmul(pin[:], identb_s[:], rh_t[:], start=False, stop=True)

                    n_t = gp.tile([P, GTILE], dt.bfloat16, tag="n")
                    nc.scalar.activation(n_t[:], pin[:], Act.Tanh)

                    t1 = gp.tile([P, GTILE], dt.bfloat16, tag="t1")
                    nc.vector.tensor_tensor(t1[:], hsv, n_t[:], op=Alu.subtract)
                    t2 = gp.tile([P, GTILE], dt.bfloat16, tag="t2")
                    nc.vector.tensor_tensor(t2[:], z_t[:], t1[:], op=Alu.mult)
                    nc.vector.tensor_tensor(
                        m_sl[:, a:a + GTILE], n_t[:], t2[:], op=Alu.add
                    )

                # ---- stats for pieces in this slab
                for pc in slab_pieces[sl]:
                    d, n, ccol = pc["d"], pc["n"], pc["col"]
                    w = ccol // WIN
                    while cur_win < w:
                        if cur_win >= 0:
                            wclose(cur_win)
                        cur_win += 1
                        wopen(cur_win)
                    mean_t, std_t, mn_t, mx_t = win_tiles
                    cw = ccol - w * WIN
                    po = pc["slot"] - s0  # offset within slab tile
                    if d == 1:
                        mseg = m_sl[:, po:po + n]
                        nc.vector.tensor_copy(mean_t[:, cw:cw + n], mseg)
                        nc.vector.tensor_copy(mn_t[:, cw:cw + n], mseg)
                        nc.vector.tensor_copy(mx_t[:, cw:cw + n], mseg)
                        continue
                    # ---- sum trees for s1 and s2 (pairs + odd carry)
                    ta = bnp.tile([P, WIN], dt.float32, tag="sc_a")
                    tb = bnp.tile([P, WIN], dt.float32, tag="sc_b")
                    sq = bnp.tile([P, SLAB], dt.float32, tag="sq")
                    mseg = m_sl[:, po:po + n * d]
                    nc.scalar.activation(sq[:, :n * d], mseg, Act.Square)
                    for src_ap, dest in (
                        (mseg, ta),   # s1
                        (sq[:, :n * d], tb),  # s2
                    ):
                        cur = src_ap.rearrange("p (n dd) -> p n dd", dd=d)
                        hh = d
                        sa = bnp.tile([P, SLAB // 2], dt.float32, tag="tr_e")
                        sb_ = bnp.tile([P, SLAB // 2], dt.float32, tag="tr_f")
                        while hh > 1:
                            m_ = hh // 2
                            odd = hh % 2
                            nxt = m_ + odd
                            if nxt == 1:
                                dst3 = dest[:, :n].rearrange("p (n h) -> p n h", h=1)
                            else:
                                dst3 = sa[:, :n * nxt].rearrange(
                                    "p (n h) -> p n h", h=nxt
                                )
                            nc.vector.tensor_tensor(
                                dst3[:, :, 0:m_],
                                cur[:, :, 0:2 * m_:2],
                                cur[:, :, 1:2 * m_:2],
                                op=Alu.add,
                            )
                            if odd:
                                nc.vector.tensor_copy(
                                    dst3[:, :, m_:m_ + 1], cur[:, :, 2 * m_:2 * m_ + 1]
                                )
                            cur = dst3
                            hh = nxt
                            sa, sb_ = sb_, sa
                    # mean = s1/d ; var = s2/d - mean^2 ; std = sqrt(max(var,0)+eps)
                    nc.vector.tensor_scalar_mul(
                        mean_t[:, cw:cw + n], ta[:, :n], float(1.0 / d)
                    )
                    nc.scalar.activation(ta[:, :n], mean_t[:, cw:cw + n], Act.Square)
                    nc.vector.tensor_scalar_mul(tb[:, :n], tb[:, :n], float(1.0 / d))
                    nc.vector.tensor_tensor(tb[:, :n], tb[:, :n], ta[:, :n], op=Alu.subtract)
                    nc.vector.tensor_scalar_max(tb[:, :n], tb[:, :n], 0.0)
                    nc.scalar.activation(
                        std_t[:, cw:cw + n], tb[:, :n], Act.Sqrt,
                        bias=eps_c[:, 0:1],
                    )
                    for op, outt, tga, tgb in (
                        (Alu.min, mn_t, "tr_a", "tr_b"),
                        (Alu.max, mx_t, "tr_a", "tr_b"),
                    ):
                        cur = m_sl[:, po:po + n * d].rearrange("p (n dd) -> p n dd", dd=d)
                        hh = d
                        sa = bnp.tile([P, SLAB // 2], dt.bfloat16, tag=tga)
                        sb = bnp.tile([P, SLAB // 2], dt.bfloat16, tag=tgb)
                        while hh > 1:
                            h1 = (hh + 1) // 2
                            if h1 == 1:
                                dst = outt[:, cw:cw + n]
                            else:
                                dst = sa[:, :n * h1].rearrange(
                                    "p (n h) -> p n h", h=h1
                                )
                            nc.vector.tensor_tensor(
                                dst, cur[:, :, 0:h1], cur[:, :, hh - h1:hh], op=op
                            )
                            cur = dst
                            hh = h1
                            sa, sb = sb, sa

            while cur_win < NW - 1:
                if cur_win >= 0:
                    wclose(cur_win)
                cur_win += 1
                wopen(cur_win)
            wclose(cur_win)

            # ---------------- alpha row + hidden_all
            t0 = pp.tile([BPC, TCAP], dt.float32, tag="tk0")
            nc.vector.memset(t0[:], NEG)
            tk_by_w = {}
            for tk in meta["tkpieces"]:
                tk_by_w.setdefault(tk["w"], []).append(tk)
            for w in range(NW):
                w0 = w * WIN
                mpz = mps.tile([P, WIN], dt.float32, tag="mp")
                ohv = r1.tile([BPC, WIN], dt.float32, tag="ohv")
                nc.sync.dma_start(out=ohv[:], in_=ohb[:, w0:w0 + WIN])
                lgv = r1.tile([1, WIN], dt.float32, tag="lgv2")
                nc.sync.dma_start(out=lgv[:], in_=lg1_dram[:, w0:w0 + WIN])
                nc.tensor.matmul(mpz[0:1, :], cu_acc[:, 0:1], ohv[:], start=True, stop=False)
                nc.tensor.matmul(mpz[0:1, :], ones_s[:, 0:1], lgv[:], start=False, stop=True)
                av = r1.tile([1, WIN], dt.float32, tag="av")
                nc.scalar.activation(
                    av[:], mpz[0:1, :], Act.Sigmoid,
                    scale=1.0 / TAU, bias=bsc_c[0:1, 0:1],
                )
                nc.sync.dma_start(out=out_alpha[:, w0:w0 + WIN], in_=av[:])
                if meta["do_topk"]:
                    for tk in tk_by_w.get(w, []):
                        nc.sync.dma_start(
                            out=t0[tk["lb"]:tk["lb"] + 1, tk["off"]:tk["off"] + tk["n"]],
                            in_=av[0:1, tk["col"] - w0:tk["col"] - w0 + tk["n"]],
                        )
                mpa = mps.tile([P, WIN], dt.float32, tag="mp")
                nc.tensor.matmul(mpa[:], ones_s[:], av[:], start=True, stop=True)
                hall = m1.tile([P, WIN], dt.float32, tag="hall")
                nc.vector.tensor_tensor(
                    hall[:], mpa[:], htT[:, w0:w0 + WIN], op=Alu.mult
                )
                nc.sync.dma_start(out=out_hall[:, w0:w0 + WIN], in_=hall[:])

            # ---------------- topk (batch-row layout)
            if meta["do_topk"]:
                work = pp.tile([BPC, TCAP], dt.float32, tag="tkw")
                res = pp.tile([BPC, TCAP], dt.float32, tag="tkr")
                # t0 rows were DMA-assembled from alpha chunks above;
                # add prep (cand: +bonus+1, else <=-9)
                nc.vector.tensor_tensor(t0[:], t0[:], prep_s[:], op=Alu.add)
                maxsc = pp.tile([BPC, 8], dt.float32, tag="tkm")
                src = t0
                for k_on in range(0, K_TOP, 8):
                    k_this = min(8, K_TOP - k_on)
                    nc.vector.max(maxsc[:], src[:])
                    if k_this < 8:
                        nc.vector.memset(maxsc[:, k_this:], NEG)
                    nc.vector.match_replace(work[:], maxsc[:], src[:], NEG)
                    src = work
                nc.vector.tensor_tensor(res[:], t0[:], work[:], op=Alu.subtract)
                nc.vector.tensor_scalar_min(res[:], res[:], 1.0)
                nc.sync.dma_start(out=out_keep[:], in_=res[:])
            else:
                zr = pp.tile([BPC, TCAP], dt.float32, tag="tk0")
                nc.vector.memset(zr[:], 0.0)
                nc.sync.dma_start(out=out_keep[:], in_=zr[:])

    nc.compile()
    return nc


# ================================================================ entry point


LAST_EXEC_NS = None
LAST_RESULT = None


def _install_ntff_hook():
    """Shim antenv.axon_hooks (absent in this image) so trace=True can
    capture NTFF profiles via the boot script's ctypes fallback."""
    import types, importlib.util, os
    try:
        import antenv.axon_hooks  # noqa
        return
    except ImportError:
        pass
    try:
        boot_py = "/root/.axon_site/trn_agent_boot/trn_boot.py"
        so_path = "/opt/axon/libaxon_pjrt.so"
        if not (os.path.exists(boot_py) and os.path.exists(so_path)):
            return
        spec = importlib.util.spec_from_file_location("_trn_boot_shim", boot_py)
        tb = importlib.util.module_from_spec(spec)
        spec.loader.exec_module(tb)
        hook = tb._ntff_profile_via_ctypes(so_path)
        if hook is None:
            return
        mod = types.ModuleType("antenv.axon_hooks")
        _state = {"hook": hook}
        mod.get_axon_ntff_profile_hook = lambda: _state["hook"]
        mod.set_axon_ntff_profile_hook = lambda h: _state.__setitem__("hook", h)
        import sys as _sys
        import antenv
        _sys.modules["antenv.axon_hooks"] = mod
        antenv.axon_hooks = mod
    except Exception:
        pass


def kernel(**inputs):
    global LAST_EXEC_NS, LAST_RESULT
    meta, host = _plan(inputs)
    nc = _build(meta)

    in_maps = [host["core_inputs"](c) for c in range(N_CORES)]
    import os
    trace = bool(int(os.environ.get("KERNEL_TRACE", "1")))
    if trace:
        _install_ntff_hook()
    res = run_bass_kernel_spmd(
        nc, in_maps, core_ids=list(range(N_CORES)), trace=trace
    )
    LAST_EXEC_NS = res.exec_time_ns
    LAST_RESULT = res
    results = res.results

    N = meta["N"]
    NODE_CAP = meta["NODE_CAP"]
    hidden_all = np.zeros((N, D), dtype=np.float32)
    alpha = np.zeros((N,), dtype=np.float32)
    keep = np.zeros((N,), dtype=np.int64)
    keep[host["old_idx"]] = 1

    for c in range(N_CORES):
        nid = host["node_of_col"][c]
        ok = nid >= 0
        hallT = np.asarray(results[c]["out_hall"])
        hidden_all[nid[ok]] = hallT.T[ok]
        alpha[nid[ok]] = np.asarray(results[c]["out_alpha"])[0][ok]
        if meta["do_topk"]:
            kr = np.asarray(results[c]["out_keep"])
            for bl in range(BPC):
                for sg in meta["segs"][bl]:
                    sel = kr[bl, sg["off"]:sg["off"] + sg["n"]] >= 0.5
                    cols = np.arange(sg["col"], sg["col"] + sg["n"])[sel]
                    gids = nid[cols]
                    keep[gids[gids >= 0]] = 1

    if not meta["do_topk"]:
        ent = host["node_ent"]
        keep = ((ent >= N_USER) & (ent < N_USER + N_ITEM)).astype(np.int64)

    return hidden_all, alpha, keep.astype(bool)
